# revision 7
# baseline (speedup 1.0000x reference)
"""Trainium2 Bass kernel for CalculateSLayer GNN message passing.

Computes, for adj [L, L, 2] f32 and h [L, D] f32 with A = adj.sum(-1):
    h_in[j, d]  = sum_i A[i, j] * h[i, d]   (= A.T @ h)
    h_out[i, d] = sum_j A[i, j] * h[j, d]   (= A @ h)

Sharding: core m holds rows i in [m*512, (m+1)*512) of A (for h_in) and
columns j in the same range (for h_out). Both outputs are computed as
per-core partials over the full [L, D] plane and summed on the host.

Wire format: each adjacency channel is centered (a - 0.5) and shipped as
fp8 e4m3 in two layouts (i-major and j-major), so that no on-chip
transposes are needed. h is shipped as fp8 plus an fp8 residual
(h - fp8(h)), duplicated across the 2 edge-channel planes. The device
runs DoubleRow fp8 matmuls whose two K-halves are the two edge channels
- the PE performs the edge-channel sum inside the contraction. The
centering is undone on the host by adding colsum(h) (exact, f64) to
every output row.
"""

import numpy as np
import ml_dtypes

L = 4096
D = 150
DD = 2 * D  # moving free width: [h8 | r8]
NCORES = 8
R = L // NCORES  # 512 rows/cols per core
P = 128  # partitions
IC = R // P  # 4 local chunks per core
JW = 512  # window width along the global axis
NW = L // JW  # 8 windows
TPW = JW // P  # 4 output tiles per window

F8 = ml_dtypes.float8_e4m3

_NC_CACHE = {}
LAST_RESULTS = None


def _ensure_ntff_hook():
    """Register the axon NTFF profile hook if the image's antenv lacks it."""
    import sys
    import types

    try:
        from antenv.axon_hooks import get_axon_ntff_profile_hook  # noqa: F401

        return
    except ImportError:
        pass

    mod = types.ModuleType("antenv.axon_hooks")
    _state = {"hook": None}
    mod.set_axon_ntff_profile_hook = lambda h: _state.__setitem__("hook", h)
    mod.get_axon_ntff_profile_hook = lambda: _state["hook"]
    sys.modules["antenv.axon_hooks"] = mod
    import antenv

    antenv.axon_hooks = mod

    so_path = "/opt/axon/libaxon_pjrt.so"
    try:
        from trn_agent_boot.trn_boot import _ntff_profile_via_ctypes

        hook = _ntff_profile_via_ctypes(so_path)
        if hook is not None:
            mod.set_axon_ntff_profile_hook(hook)
    except Exception:
        pass

    try:
        from concourse import bass_utils

        bass_utils.upload_artifacts = lambda tmpdir: tmpdir
    except Exception:
        pass


def _build_nc():
    import concourse.bacc as bacc
    import concourse.tile as tile
    import concourse.mybir as mybir

    f8 = mybir.dt.float8e4
    f32 = mybir.dt.float32
    bf16 = mybir.dt.bfloat16
    DR = mybir.MatmulPerfMode.DoubleRow

    nc = bacc.Bacc(
        "TRN2", target_bir_lowering=False, debug=False, num_devices=NCORES
    )
    # centered fp8 channels: row block (i-major) and column block (j-major),
    # pre-tiled as [partition, local-chunk, channel, global] so a window
    # load is a 3-dim DMA pattern
    bch_d = nc.dram_tensor("bch", [P, IC, 2, L], f8, kind="ExternalInput").ap()
    bcht_d = nc.dram_tensor("bcht", [P, IC, 2, L], f8, kind="ExternalInput").ap()
    # this core's h rows as [h8 | r8], duplicated over the channel axis
    hb_d = nc.dram_tensor("hbdup", [P, IC, 2, DD], f8, kind="ExternalInput").ap()
    # per-core partials of both outputs, full [L, D] plane each
    pin_d = nc.dram_tensor("pin", [L, D], bf16, kind="ExternalOutput").ap()
    hout_d = nc.dram_tensor("hout", [L, D], bf16, kind="ExternalOutput").ap()

    pin_r = pin_d.rearrange("(w t p) d -> p w t d", p=P, t=TPW)
    hout_r = hout_d.rearrange("(w t p) d -> p w t d", p=P, t=TPW)

    with tile.TileContext(nc) as tc:
        with (
            tc.tile_pool(name="const", bufs=1) as const_pool,
            tc.tile_pool(name="adj", bufs=2) as adj_pool,
            tc.tile_pool(name="stage", bufs=2) as stage_pool,
            tc.tile_pool(name="ps", bufs=1, space="PSUM") as psum_pool,
        ):
            hb_sb = const_pool.tile([P, IC, 2, DD], f8)
            nc.gpsimd.dma_start(hb_sb[:], hb_d)

            for w in range(NW):
                j0 = w * JW
                a_sb = adj_pool.tile([P, IC, 2, JW], f8, tag="a", name="a_sb")
                at_sb = adj_pool.tile([P, IC, 2, JW], f8, tag="at", name="at_sb")
                nc.sync.dma_start(a_sb[:], bch_d[:, :, :, j0 : j0 + JW])
                nc.sync.dma_start(at_sb[:], bcht_d[:, :, :, j0 : j0 + JW])

                pin_sb = stage_pool.tile([P, TPW, D], bf16, tag="pins",
                                         name="pin_sb")
                po_sb = stage_pool.tile([P, TPW, D], bf16, tag="pos",
                                        name="po_sb")

                for t in range(TPW):
                    ppin = psum_pool.tile([P, 512], f32, tag=f"pin{t}",
                                          name=f"ppin{t}")
                    ppo = psum_pool.tile([P, 512], f32, tag=f"po{t}",
                                         name=f"ppo{t}")
                    for k in range(IC):
                        nc.tensor.matmul(
                            ppin[:, 0:DD],
                            a_sb[:, k, :, t * P : (t + 1) * P],
                            hb_sb[:, k, :, :],
                            start=(k == 0),
                            stop=(k == IC - 1),
                            perf_mode=DR,
                        )
                    for k in range(IC):
                        nc.tensor.matmul(
                            ppo[:, 0:DD],
                            at_sb[:, k, :, t * P : (t + 1) * P],
                            hb_sb[:, k, :, :],
                            start=(k == 0),
                            stop=(k == IC - 1),
                            perf_mode=DR,
                        )
                    # fold the residual half into the h8 half, cast to bf16
                    # (single-input PSUM read: pair axis innermost, reduced)
                    with nc.allow_low_precision(reason="2-term fold to bf16"):
                        nc.vector.tensor_reduce(
                            pin_sb[:, t, :],
                            ppin[:, 0:DD].rearrange("p (two d) -> p d two", two=2),
                            axis=mybir.AxisListType.X,
                            op=mybir.AluOpType.add,
                        )
                        nc.vector.tensor_reduce(
                            po_sb[:, t, :],
                            ppo[:, 0:DD].rearrange("p (two d) -> p d two", two=2),
                            axis=mybir.AxisListType.X,
                            op=mybir.AluOpType.add,
                        )

                nc.scalar.dma_start(pin_r[:, w], pin_sb[:])
                nc.scalar.dma_start(hout_r[:, w], po_sb[:])

    nc.compile()
    return nc


def _get_nc():
    if "nc" not in _NC_CACHE:
        _NC_CACHE["nc"] = _build_nc()
    return _NC_CACHE["nc"]


def _prep_inputs(adj, h):
    """Quantize + shard on the host; returns per-core input dicts."""
    b8 = (adj - np.float32(0.5)).astype(F8)  # [L, L, 2] centered channels
    h8 = h.astype(F8)
    r8 = (h - h8.astype(np.float32)).astype(F8)
    hd = np.concatenate([h8, r8], axis=1)  # [L, DD]

    in_maps = []
    for m in range(NCORES):
        rows = b8[m * R : (m + 1) * R]  # [R, L, 2]
        cols = b8[:, m * R : (m + 1) * R, :]  # [L, R, 2]
        # [P, IC, 2, L]: [partition, local chunk, channel, global axis]
        bch = np.ascontiguousarray(
            rows.reshape(IC, P, L, 2).transpose(1, 0, 3, 2)
        )
        bcht = np.ascontiguousarray(
            cols.transpose(1, 2, 0).reshape(IC, P, 2, L).transpose(1, 0, 2, 3)
        )
        blk = hd[m * R : (m + 1) * R].reshape(IC, P, DD).transpose(1, 0, 2)
        hbdup = np.ascontiguousarray(
            np.broadcast_to(blk[:, :, None, :], (P, IC, 2, DD))
        )
        in_maps.append({"bch": bch, "bcht": bcht, "hbdup": hbdup})
    return in_maps


def _run_cores(adj, h, trace=False):
    from concourse.bass_utils import run_bass_kernel_spmd

    global LAST_RESULTS
    if trace:
        _ensure_ntff_hook()
    nc = _get_nc()
    in_maps = _prep_inputs(adj, h)
    res = run_bass_kernel_spmd(
        nc, in_maps, core_ids=list(range(NCORES)), trace=trace
    )
    LAST_RESULTS = res
    return res


def kernel(unpreprocessed_unweight_adj_matrix, h):
    adj = np.ascontiguousarray(
        np.asarray(unpreprocessed_unweight_adj_matrix, dtype=np.float32)
    )
    h = np.ascontiguousarray(np.asarray(h, dtype=np.float32))
    res = _run_cores(adj, h)
    parts = res.results

    colsum = h.astype(np.float64).sum(axis=0)  # undo the -0.5 centering
    h_in = np.zeros((L, D), dtype=np.float64)
    h_out = np.zeros((L, D), dtype=np.float64)
    for r in parts:
        h_in += np.asarray(r["pin"], dtype=np.float32).astype(np.float64)
        h_out += np.asarray(r["hout"], dtype=np.float32).astype(np.float64)
    h_in += colsum[None, :]
    h_out += colsum[None, :]
    return (
        np.ascontiguousarray(h_in, dtype=np.float32),
        np.ascontiguousarray(h_out, dtype=np.float32),
    )


# revision 14
# speedup vs baseline: 1.4628x; 1.4628x over previous
"""Trainium2 Bass kernel for CalculateSLayer GNN message passing.

Computes, for adj [L, L, 2] f32 and h [L, D] f32 with A = adj.sum(-1):
    h_in[j, d]  = sum_i A[i, j] * h[i, d]   (= A.T @ h)
    h_out[i, d] = sum_j A[i, j] * h[j, d]   (= A @ h)

Sharding: core m holds rows i in [m*512, (m+1)*512) of A (for h_in) and
columns j in the same range (for h_out). Both outputs are computed as
per-core partials over the full [L, D] plane and summed on the host.

Wire format: each adjacency channel is centered (a - 0.5) and shipped as
fp8 e4m3 in two layouts (i-major and j-major), pre-tiled per 512-wide
window so every DMA row is one contiguous 4KB run. h is shipped as fp8
plus an fp8 residual (h - fp8(h)), duplicated across the 2 edge-channel
planes. The device runs DoubleRow fp8 matmuls with the h-chunks as the
stationary operand and the adjacency window tiles as the 512-wide moving
operand; the two K-halves are the two edge channels, so the PE performs
the edge-channel sum inside the contraction, and the h8/r8 residual pair
accumulates into the same PSUM tile (no separate fold pass). Outputs
leave the chip d-major (transposed); the host transposes back, folds the
d-tail halves, and undoes the centering by adding colsum(h) (exact, f64)
to every output row.
"""

import numpy as np
import ml_dtypes

L = 4096
D = 150
DD = 2 * D  # [h8 | r8] width
HBW = 512  # padded h-block plane width (ldweights stride alignment)
DT0 = 128  # main d-tile
DT1 = D - DT0  # 22-wide d-tail
NCORES = 8
R = L // NCORES  # 512 rows/cols per core
P = 128  # partitions
IC = R // P  # 4 local chunks per core
JW = 512  # window width along the global axis
NW = L // JW  # 8 windows

F8 = ml_dtypes.float8_e4m3

_NC_CACHE = {}
LAST_RESULTS = None


def _ensure_ntff_hook():
    """Register the axon NTFF profile hook if the image's antenv lacks it."""
    import sys
    import types

    try:
        from antenv.axon_hooks import get_axon_ntff_profile_hook  # noqa: F401

        return
    except ImportError:
        pass

    mod = types.ModuleType("antenv.axon_hooks")
    _state = {"hook": None}
    mod.set_axon_ntff_profile_hook = lambda h: _state.__setitem__("hook", h)
    mod.get_axon_ntff_profile_hook = lambda: _state["hook"]
    sys.modules["antenv.axon_hooks"] = mod
    import antenv

    antenv.axon_hooks = mod

    so_path = "/opt/axon/libaxon_pjrt.so"
    try:
        from trn_agent_boot.trn_boot import _ntff_profile_via_ctypes

        hook = _ntff_profile_via_ctypes(so_path)
        if hook is not None:
            mod.set_axon_ntff_profile_hook(hook)
    except Exception:
        pass

    try:
        from concourse import bass_utils

        bass_utils.upload_artifacts = lambda tmpdir: tmpdir
    except Exception:
        pass


def _build_nc():
    import concourse.bacc as bacc
    import concourse.tile as tile
    import concourse.mybir as mybir

    f8 = mybir.dt.float8e4
    f32 = mybir.dt.float32
    bf16 = mybir.dt.bfloat16
    DR = mybir.MatmulPerfMode.DoubleRow

    nc = bacc.Bacc(
        "TRN2", target_bir_lowering=False, debug=False, num_devices=NCORES
    )
    # centered fp8 channels, pre-windowed: [P, window, chunk, channel, JW]
    aw_d = nc.dram_tensor("aw", [P, NW, IC, 2, JW], f8, kind="ExternalInput").ap()
    atw_d = nc.dram_tensor("atw", [P, NW, IC, 2, JW], f8,
                           kind="ExternalInput").ap()
    # this core's h rows as [h8 | r8], duplicated over the channel axis
    hb_d = nc.dram_tensor("hbdup", [P, IC, 2, HBW], f8, kind="ExternalInput").ap()
    # outputs, d-major: rows 0:128 of h_inT / h_outT partials, and the
    # unfolded 44-row tail (h8-part rows 0:22, r8-part rows 22:44)
    pin0_d = nc.dram_tensor("pin0", [P, L], bf16, kind="ExternalOutput").ap()
    pin1_d = nc.dram_tensor("pin1", [2 * DT1, L], bf16,
                            kind="ExternalOutput").ap()
    hout0_d = nc.dram_tensor("hout0", [P, L], bf16, kind="ExternalOutput").ap()
    hout1_d = nc.dram_tensor("hout1", [2 * DT1, L], bf16,
                             kind="ExternalOutput").ap()

    with tile.TileContext(nc) as tc:
        with (
            tc.tile_pool(name="const", bufs=1) as const_pool,
            tc.tile_pool(name="adj", bufs=3) as adj_pool,
            tc.tile_pool(name="stage", bufs=2) as stage_pool,
            tc.tile_pool(name="ps", bufs=2, space="PSUM") as psum_pool,
        ):
            # columns: [h8 d0:128 | r8 d0:128 | h8 d128:150 | r8 d128:150]
            hb_sb = const_pool.tile([P, IC, 2, HBW], f8)
            nc.gpsimd.dma_start(hb_sb[:], hb_d)

            outs = (
                ("pin", aw_d, pin0_d, pin1_d),
                ("po", atw_d, hout0_d, hout1_d),
            )

            for w in range(NW):
                j0 = w * JW
                a_sb = adj_pool.tile([P, IC, 2, JW], f8, tag="a", name="a_sb")
                at_sb = adj_pool.tile([P, IC, 2, JW], f8, tag="at",
                                      name="at_sb")
                nc.sync.dma_start(a_sb[:], aw_d[:, w])
                nc.sync.dma_start(at_sb[:], atw_d[:, w])

                for g, (gname, _, out0_d, out1_d) in enumerate(outs):
                    mov = a_sb if g == 0 else at_sb
                    t0 = psum_pool.tile([P, JW], f32, tag=f"t0{gname}",
                                        name=f"t0{gname}")
                    t1 = psum_pool.tile([2 * DT1, JW], f32, tag=f"t1{gname}",
                                        name=f"t1{gname}")
                    # main d-tile: h8 and r8 halves accumulate into the
                    # same PSUM bank (residual folded by the PE)
                    for half in range(2):
                        d0 = half * DT0
                        for k in range(IC):
                            nc.tensor.matmul(
                                t0[:],
                                hb_sb[:, k, :, d0 : d0 + DT0],
                                mov[:, k],
                                start=(half == 0 and k == 0),
                                stop=(half == 1 and k == IC - 1),
                                perf_mode=DR,
                            )
                    # d-tail: both halves side by side as 44 output rows
                    for k in range(IC):
                        nc.tensor.matmul(
                            t1[:],
                            hb_sb[:, k, :, 2 * DT0 : DD],
                            mov[:, k],
                            start=(k == 0),
                            stop=(k == IC - 1),
                            perf_mode=DR,
                        )
                    s0 = stage_pool.tile([P, JW], bf16, tag=f"s0{gname}",
                                         name=f"s0{gname}")
                    s1 = stage_pool.tile([2 * DT1, JW], bf16,
                                         tag=f"s1{gname}", name=f"s1{gname}")
                    nc.scalar.copy(s0[:], t0[:])
                    nc.vector.tensor_copy(s1[:], t1[:])
                    nc.gpsimd.dma_start(out0_d[:, j0 : j0 + JW], s0[:])
                    nc.gpsimd.dma_start(out1_d[:, j0 : j0 + JW], s1[:])

    nc.compile()
    return nc


def _get_nc():
    if "nc" not in _NC_CACHE:
        _NC_CACHE["nc"] = _build_nc()
    return _NC_CACHE["nc"]


def _prep_inputs(adj, h):
    """Quantize + shard on the host; returns per-core input dicts."""
    b8 = (adj - np.float32(0.5)).astype(F8)  # [L, L, 2] centered channels
    h8 = h.astype(F8)
    r8 = (h - h8.astype(np.float32)).astype(F8)
    # [h8 d0:128 | r8 d0:128 | h8 d128:150 | r8 d128:150]
    hd = np.zeros((L, HBW), dtype=F8)
    hd[:, 0:DT0] = h8[:, :DT0]
    hd[:, DT0 : 2 * DT0] = r8[:, :DT0]
    hd[:, 2 * DT0 : DD] = np.concatenate([h8[:, DT0:], r8[:, DT0:]], axis=1)

    in_maps = []
    for m in range(NCORES):
        rows = b8[m * R : (m + 1) * R]  # [R, L, 2] = [i_local, j, c]
        cols = b8[:, m * R : (m + 1) * R, :]  # [L, R, 2] = [i, j_local, c]
        # [P, NW, IC, 2, JW]
        aw = np.ascontiguousarray(
            rows.reshape(IC, P, NW, JW, 2).transpose(1, 2, 0, 4, 3)
        )
        atw = np.ascontiguousarray(
            cols.transpose(1, 2, 0).reshape(IC, P, 2, NW, JW)
            .transpose(1, 3, 0, 2, 4)
        )
        blk = hd[m * R : (m + 1) * R].reshape(IC, P, HBW).transpose(1, 0, 2)
        hbdup = np.ascontiguousarray(
            np.broadcast_to(blk[:, :, None, :], (P, IC, 2, HBW))
        )
        in_maps.append({"aw": aw, "atw": atw, "hbdup": hbdup})
    return in_maps


def _run_cores(adj, h, trace=False):
    from concourse.bass_utils import run_bass_kernel_spmd

    global LAST_RESULTS
    if trace:
        _ensure_ntff_hook()
    nc = _get_nc()
    in_maps = _prep_inputs(adj, h)
    res = run_bass_kernel_spmd(
        nc, in_maps, core_ids=list(range(NCORES)), trace=trace
    )
    LAST_RESULTS = res
    return res


def kernel(unpreprocessed_unweight_adj_matrix, h):
    adj = np.ascontiguousarray(
        np.asarray(unpreprocessed_unweight_adj_matrix, dtype=np.float32)
    )
    h = np.ascontiguousarray(np.asarray(h, dtype=np.float32))
    res = _run_cores(adj, h)
    parts = res.results

    colsum = h.astype(np.float64).sum(axis=0)  # undo the -0.5 centering
    h_inT = np.zeros((D, L), dtype=np.float64)
    h_outT = np.zeros((D, L), dtype=np.float64)
    for r in parts:
        for t_acc, k0, k1 in ((h_inT, "pin0", "pin1"), (h_outT, "hout0", "hout1")):
            p0 = np.asarray(r[k0], dtype=np.float32)
            p1 = np.asarray(r[k1], dtype=np.float32)
            t_acc[0:DT0] += p0
            t_acc[DT0:D] += p1[0:DT1] + p1[DT1 : 2 * DT1]
    return _finalize(h_inT, h_outT, colsum)


def _finalize(h_inT, h_outT, colsum):
    h_in = h_inT.T + colsum[None, :]
    h_out = h_outT.T + colsum[None, :]
    return (
        np.ascontiguousarray(h_in, dtype=np.float32),
        np.ascontiguousarray(h_out, dtype=np.float32),
    )
